# revision 3
# baseline (speedup 1.0000x reference)
"""Trainium2 Bass kernel for nn_AutoEncoder_1700807049822.

Data-parallel over batch: 8 cores x 32 batches. Per core:
  X [s=128, (b=32, e=768)] f32 in SBUF (12.6 MB), loaded in 8 groups of 4 batches.
  Q mean      : col-tiled f32 matmuls (ones lhsT, M=32-replicated rows)
  q transpose : PE transposes (fp16)
  QW = qW     : fp16 matmul per supergroup of 8 batches (W_att streamed)
  replicate   : K=1 row-tiled fp16 matmuls -> qW broadcast in PSUM
  scores      : DVE scalar_tensor_tensor fused mult+reduce (f32)
  softmax     : exp on ACT (no max-subtract; scores are O(10)), colsum via
                ones-matmul, normalization folded into the z PSUM evacuation
  z           : col-tiled f32 matmuls (exp-weights, M=32-replicated)
  comp/recon  : small f32 matmuls; loss tail with a zT AllGather and a
                host-prepared negative-count mask (handles duplicate indices)
Outputs per core: z rows, recon rows, margin partial sum, reg. Host glue sums
margin partials and adds reg.
"""

import numpy as np

import concourse.bass as bass
import concourse.tile as tile
from concourse import bacc, mybir
from concourse import bass_utils

F32 = mybir.dt.float32
F16 = mybir.dt.float16

N_CORES = 8
B, S, E, A = 256, 128, 768, 30
BL = B // N_CORES          # 32 local batches
NG = 8                     # batch groups per core
GB = BL // NG              # 4 batches per group
NC_E = E // 128            # 6 e-chunks
AluOp = mybir.AluOpType
Act = mybir.ActivationFunctionType

_CACHED = {}


def build_program():
    nc = bacc.Bacc("TRN2", target_bir_lowering=False, debug=False,
                   num_devices=N_CORES)

    # ---- DRAM I/O ----
    x_d = nc.dram_tensor("x_loc", [BL, S, E], F32, kind="ExternalInput").ap()
    w16_d = nc.dram_tensor("w_att16", [E, E], F16, kind="ExternalInput").ap()
    batt16_d = nc.dram_tensor("b_att16", [1, E], F16, kind="ExternalInput").ap()
    wred_d = nc.dram_tensor("w_red", [E, A], F32, kind="ExternalInput").ap()
    bred_d = nc.dram_tensor("b_red_row", [1, A], F32, kind="ExternalInput").ap()
    asp_d = nc.dram_tensor("aspect_w", [A, E], F32, kind="ExternalInput").ap()
    mt_d = nc.dram_tensor("m_t", [BL, B], F32, kind="ExternalInput").ap()
    onesq_d = nc.dram_tensor("ones_q32", [128, 32], F32, kind="ExternalInput").ap()
    onesf_d = nc.dram_tensor("ones_f32", [128, 128], F32, kind="ExternalInput").ap()
    ones16_d = nc.dram_tensor("ones_16", [128, 128], F16, kind="ExternalInput").ap()
    i32_d = nc.dram_tensor("ident_f32", [128, 128], F32, kind="ExternalInput").ap()
    i16_d = nc.dram_tensor("ident_16", [128, 128], F16, kind="ExternalInput").ap()

    z_out = nc.dram_tensor("z_loc", [BL, E], F32, kind="ExternalOutput").ap()
    rc_out = nc.dram_tensor("recon_loc", [BL, E], F32, kind="ExternalOutput").ap()
    mg_out = nc.dram_tensor("margin_loc", [1, 1], F32, kind="ExternalOutput").ap()
    rg_out = nc.dram_tensor("reg_out", [1, 1], F32, kind="ExternalOutput").ap()

    with tile.TileContext(nc) as tc:
        with (
            tc.tile_pool(name="xp", bufs=NG) as xp,
            tc.tile_pool(name="cst", bufs=1) as cst,
            tc.tile_pool(name="wk", bufs=2) as wk,
            tc.tile_pool(name="prs", bufs=1) as prs,
            tc.tile_pool(name="mm", bufs=2, space="PSUM") as mmp,
            tc.tile_pool(name="qb", bufs=2, space="PSUM") as qbp,
            tc.tile_pool(name="dram", bufs=1, space="DRAM") as dram,
        ):
            # ---- constants / weights ----
            w16 = cst.tile([128, NC_E * E], F16, tag="w16")
            nc.sync.dma_start(
                out=w16[:],
                in_=w16_d.rearrange("(k p) e -> p k e", p=128),
            )
            batt16 = cst.tile([1, E], F16, tag="batt16")
            nc.sync.dma_start(out=batt16[:], in_=batt16_d)
            wred = cst.tile([128, NC_E * A], F32, tag="wred")
            nc.sync.dma_start(
                out=wred[:], in_=wred_d.rearrange("(k p) a -> p k a", p=128)
            )
            bred = cst.tile([1, A], F32, tag="bred")
            nc.sync.dma_start(out=bred[:], in_=bred_d)
            asp = cst.tile([A, E], F32, tag="asp")
            nc.sync.dma_start(out=asp[:], in_=asp_d)
            mt = cst.tile([BL, B], F32, tag="mt")
            nc.sync.dma_start(out=mt[:], in_=mt_d)
            onesq = cst.tile([128, 32], F32, tag="onesq")
            nc.sync.dma_start(out=onesq[:], in_=onesq_d)
            onesf = cst.tile([128, 128], F32, tag="onesf")
            nc.sync.dma_start(out=onesf[:], in_=onesf_d)
            ones16 = cst.tile([128, 128], F16, tag="ones16")
            nc.sync.dma_start(out=ones16[:], in_=ones16_d)
            ident = cst.tile([128, 128], F32, tag="ident")
            nc.sync.dma_start(out=ident[:], in_=i32_d)
            ident16 = cst.tile([128, 128], F16, tag="ident16")
            nc.sync.dma_start(out=ident16[:], in_=i16_d)

            # ---- persistent working tiles ----
            scoresT = prs.tile([128, BL], F32, tag="scoresT")
            exp_sT = prs.tile([128, BL], F32, tag="exp_sT")
            recip_row = prs.tile([1, BL], F32, tag="recip_row")
            zT = prs.tile([128, NC_E * BL], F32, tag="zT")
            z_bp = prs.tile([BL, E], F32, tag="z_bp")
            scr = prs.tile([128, E], F32, tag="scr")        # STT dump
            scr2 = prs.tile([128, E], F32, tag="scr2")      # misc dump
            tnt = prs.tile([128, NC_E * A], F32, tag="tnt")  # T_n^T

            # ---- X loads (one tile per group for precise deps) ----
            xg = []
            for g in range(NG):
                xt = xp.tile([128, GB * E], F32, tag="x")
                nc.sync.dma_start(
                    out=xt[:],
                    in_=x_d[g * GB : (g + 1) * GB].rearrange("b s e -> s b e"),
                )
                xg.append(xt)

            # =========== regularizer (independent; overlaps DMA) ===========
            ss_a = prs.tile([A, 1], F32, tag="ss_a")
            nc.scalar.activation(out=scr2[0:A, :], in_=asp[:], func=Act.Square,
                                 accum_out=ss_a[:])
            inv_a = prs.tile([A, 1], F32, tag="inv_a")
            nc.vector.reciprocal(inv_a[:], ss_a[:])
            nc.scalar.activation(out=inv_a[:], in_=inv_a[:], func=Act.Sqrt)
            tn = prs.tile([A, E], F32, tag="tn")
            nc.vector.tensor_scalar_mul(tn[:], asp[:], inv_a[:])
            for c in range(NC_E):
                tp = mmp.tile([128, A], F32, tag="mm")
                nc.tensor.transpose(tp[:], tn[:, c * 128 : (c + 1) * 128],
                                    ident[0:A, 0:A])
                nc.scalar.copy(tnt[:, c * A : (c + 1) * A], tp[:])
            gps = mmp.tile([A, A], F32, tag="mm")
            for c in range(NC_E):
                nc.tensor.matmul(
                    out=gps[:],
                    lhsT=tnt[:, c * A : (c + 1) * A],
                    rhs=tnt[:, c * A : (c + 1) * A],
                    start=(c == 0),
                    stop=(c == NC_E - 1),
                )
            gm = prs.tile([A, A], F32, tag="gm")
            nc.vector.tensor_sub(gm[:], gps[:], ident[0:A, 0:A])
            grow = prs.tile([A, 1], F32, tag="grow")
            nc.vector.scalar_tensor_tensor(
                out=scr2[0:A, 0:A], in0=gm[:], scalar=1.0, in1=gm[:],
                op0=AluOp.mult, op1=AluOp.mult, accum_out=grow[:],
            )
            rps = mmp.tile([1, 1], F32, tag="mm")
            nc.tensor.matmul(out=rps[:], lhsT=onesf[0:A, 0:1], rhs=grow[:],
                             start=True, stop=True)
            reg_sb = prs.tile([1, 1], F32, tag="reg_sb")
            nc.scalar.activation(out=reg_sb[:], in_=rps[:], func=Act.Sqrt)
            nc.sync.dma_start(out=rg_out, in_=reg_sb[:])

            # =========== main pipeline ===========
            NSG = NG // 2  # supergroups of 8 batches
            for sg in range(NSG):
                qt16 = wk.tile([128, NC_E * 2 * GB], F16, tag="qt16")
                for t in range(2):
                    g = 2 * sg + t
                    # --- Q mean: col-tiled, M=32 replicated rows ---
                    qps = mmp.tile([128, E], F32, tag="mm")
                    for j in range(GB):
                        for (o, n) in ((0, 512), (512, 256)):
                            nc.tensor.matmul(
                                out=qps[32 * j : 32 * (j + 1), o : o + n],
                                lhsT=onesq[:],
                                rhs=xg[g][:, j * E + o : j * E + o + n],
                                start=True, stop=True,
                                tile_position=(0, 32 * j),
                            )
                    q16 = wk.tile([128, E], F16, tag="q16")
                    nc.scalar.copy(q16[:], qps[:])
                    # --- q transposes (fp16) ---
                    qtp = mmp.tile([128, E], F16, tag="mm")
                    for c in range(NC_E):
                        nc.tensor.transpose(
                            qtp[:, c * 128 : (c + 1) * 128],
                            q16[:, c * 128 : (c + 1) * 128],
                            ident16[:],
                        )
                    # evac cols {0,32,64,96} of each chunk
                    nc.scalar.copy(
                        qt16[:].rearrange("p (c b) -> p c b", b=2 * GB)[:, :, t * GB:(t + 1) * GB],
                        qtp[:].rearrange("p (c x) -> p c x", x=128)[:, :, ::32],
                    )
                # --- QW for the supergroup (fp16) ---
                qwps = mmp.tile([8, E], F32, tag="mm")
                for c in range(NC_E):
                    for (o, n) in ((0, 512), (512, 256)):
                        nc.tensor.matmul(
                            out=qwps[:, o : o + n],
                            lhsT=qt16[:].rearrange("p (c b) -> p c b", b=2 * GB)[:, c, :],
                            rhs=w16[:, c * E + o : c * E + o + n],
                            start=(c == 0), stop=False,
                        )
                for (o, n) in ((0, 512), (512, 256)):
                    nc.tensor.matmul(out=qwps[:, o : o + n],
                                     lhsT=ones16[0:1, 0:8],
                                     rhs=batt16[:, o : o + n],
                                     start=False, stop=True)
                qw8 = wk.tile([8, E], F16, tag="qw8")
                nc.scalar.copy(qw8[:], qwps[:])
                qw_sp = wk.tile([128, 2 * E], F16, tag="qw_sp")
                for t in range(2):
                    nc.sync.dma_start(
                        out=qw_sp[::32, t * E : (t + 1) * E],
                        in_=qw8[GB * t : GB * (t + 1), :],
                    )
                # --- replicate + scores per batch ---
                for t in range(2):
                    g = 2 * sg + t
                    for j in range(GB):
                        b = g * GB + j
                        qbps = qbp.tile([128, E], F32, tag="qb")
                        for (o, n) in ((0, 512), (512, 256)):
                            nc.tensor.matmul(
                                out=qbps[:, o : o + n],
                                lhsT=ones16[32 * j : 32 * j + 1, :],
                                rhs=qw_sp[32 * j : 32 * j + 1,
                                          t * E + o : t * E + o + n],
                                start=True, stop=True,
                                tile_position=(32 * j, 0),
                            )
                        nc.vector.scalar_tensor_tensor(
                            out=scr[:],
                            in0=xg[g][:, j * E : (j + 1) * E],
                            scalar=1.0,
                            in1=qbps[:],
                            op0=AluOp.mult, op1=AluOp.mult,
                            accum_out=scoresT[:, b : b + 1],
                        )
                # --- softmax + z per group ---
                for t in range(2):
                    g = 2 * sg + t
                    gc = slice(g * GB, (g + 1) * GB)
                    nc.scalar.activation(out=exp_sT[:, gc], in_=scoresT[:, gc],
                                         func=Act.Exp)
                    csps = mmp.tile([1, GB], F32, tag="mm")
                    nc.tensor.matmul(out=csps[:], lhsT=onesf[:, 0:1],
                                     rhs=exp_sT[:, gc], start=True, stop=True)
                    nc.vector.reciprocal(recip_row[:, gc], csps[:])
                    sc_col = wk.tile([128, 1], F32, tag="sc_col")
                    nc.vector.memset(sc_col[:], 1.0)
                    nc.sync.dma_start(out=sc_col[::32, :],
                                      in_=recip_row[0:1, gc])
                    exp_rep = wk.tile([128, GB, 32], F32, tag="exp_rep")
                    nc.vector.tensor_copy(
                        exp_rep[:],
                        exp_sT[:, gc].unsqueeze(2).broadcast_to([128, GB, 32]),
                    )
                    zps = mmp.tile([128, E], F32, tag="mm")
                    for j in range(GB):
                        for (o, n) in ((0, 512), (512, 256)):
                            nc.tensor.matmul(
                                out=zps[32 * j : 32 * (j + 1), o : o + n],
                                lhsT=exp_rep[:, j, :],
                                rhs=xg[g][:, j * E + o : j * E + o + n],
                                start=True, stop=True,
                                tile_position=(0, 32 * j),
                            )
                    z_sb = wk.tile([128, E], F32, tag="z_sb")
                    nc.scalar.activation(out=z_sb[:], in_=zps[:], func=Act.Copy,
                                         scale=sc_col[:])
                    nc.sync.dma_start(out=z_out[gc, :], in_=z_sb[::32, :])
                    nc.sync.dma_start(out=z_bp[gc, :], in_=z_sb[::32, :])
                    ztp = mmp.tile([128, E], F32, tag="mm")
                    for c in range(NC_E):
                        nc.tensor.transpose(
                            ztp[:, c * 128 : (c + 1) * 128],
                            z_sb[:, c * 128 : (c + 1) * 128],
                            ident[:],
                        )
                    nc.scalar.copy(
                        zT[:].rearrange("p (c b) -> p c b", b=BL)[:, :, gc],
                        ztp[:].rearrange("p (c x) -> p c x", x=128)[:, :, ::32],
                    )

            # =========== loss tail ===========
            # comp logits = z @ W_red + b_red   -> [BL, A] (b on partitions)
            cpps = mmp.tile([BL, A], F32, tag="mm")
            for c in range(NC_E):
                nc.tensor.matmul(
                    out=cpps[:],
                    lhsT=zT[:, c * BL : (c + 1) * BL],
                    rhs=wred[:, c * A : (c + 1) * A],
                    start=(c == 0), stop=False,
                )
            nc.tensor.matmul(out=cpps[:], lhsT=onesf[0:1, 0:BL], rhs=bred[:],
                             start=False, stop=True)
            exp_cp = prs.tile([BL, A], F32, tag="exp_cp")
            den = prs.tile([BL, 1], F32, tag="den")
            nc.scalar.activation(out=exp_cp[:], in_=cpps[:], func=Act.Exp,
                                 accum_out=den[:])
            rden = prs.tile([BL, 1], F32, tag="rden")
            nc.vector.reciprocal(rden[:], den[:])
            # compT via PE transpose
            ctps = mmp.tile([A, BL], F32, tag="mm")
            nc.tensor.transpose(ctps[:], exp_cp[:], ident[0:BL, 0:BL])
            compT = prs.tile([A, BL], F32, tag="compT")
            nc.scalar.copy(compT[:], ctps[:])
            # recon rows (scale by 1/den at evac)
            rcps = mmp.tile([BL, E], F32, tag="mm")
            for (o, n) in ((0, 512), (512, 256)):
                nc.tensor.matmul(
                    out=rcps[:, o : o + n],
                    lhsT=compT[:],
                    rhs=asp[:, o : o + n],
                    start=True, stop=True,
                )
            recon_sb = prs.tile([BL, E], F32, tag="recon_sb")
            nc.scalar.activation(out=recon_sb[:], in_=rcps[:], func=Act.Copy,
                                 scale=rden[:])
            nc.sync.dma_start(out=rc_out, in_=recon_sb[:])
            # r_s = recon / ||recon||
            ssr = prs.tile([BL, 1], F32, tag="ssr")
            nc.scalar.activation(out=scr2[0:BL, :], in_=recon_sb[:],
                                 func=Act.Square, accum_out=ssr[:])
            isq = prs.tile([BL, 1], F32, tag="isq")
            nc.vector.reciprocal(isq[:], ssr[:])
            nc.scalar.activation(out=isq[:], in_=isq[:], func=Act.Sqrt)
            rsc = prs.tile([BL, E], F32, tag="rsc")
            nc.vector.tensor_scalar_mul(rsc[:], recon_sb[:], isq[:])
            # pos = <z, r_s> ; om_pos = 1 - pos
            pos = prs.tile([BL, 1], F32, tag="pos")
            nc.vector.scalar_tensor_tensor(
                out=scr[0:BL, :], in0=z_bp[:], scalar=1.0, in1=rsc[:],
                op0=AluOp.mult, op1=AluOp.mult, accum_out=pos[:],
            )
            om_pos = prs.tile([BL, 1], F32, tag="om_pos")
            nc.vector.tensor_scalar(
                out=om_pos[:], in0=pos[:], scalar1=-1.0, scalar2=1.0,
                op0=AluOp.mult, op1=AluOp.add,
            )
            # r_s^T via PE transposes
            rst = prs.tile([128, NC_E * BL], F32, tag="rst")
            rtps = mmp.tile([128, BL], F32, tag="mm")
            for c in range(NC_E):
                nc.tensor.transpose(rtps[:], rsc[:, c * 128 : (c + 1) * 128],
                                    ident[0:BL, 0:BL])
                nc.scalar.copy(rst[:, c * BL : (c + 1) * BL], rtps[:])
            # AllGather zT
            zt_in = dram.tile([128, NC_E * BL], F32)
            zt_all = dram.tile([N_CORES, 128, NC_E * BL], F32)
            nc.sync.dma_start(out=zt_in[:], in_=zT[:])
            nc.gpsimd.collective_compute(
                "AllGather", AluOp.bypass,
                replica_groups=[list(range(N_CORES))],
                ins=[zt_in[:].opt()], outs=[zt_all[:].opt()],
            )
            ztf = prs.tile([128, NC_E * B], F32, tag="ztf")
            for cc in range(N_CORES):
                nc.sync.dma_start(
                    out=ztf[:].rearrange("p (c j) -> p c j", j=B)[:, :, cc * BL:(cc + 1) * BL],
                    in_=zt_all[cc],
                )
            # D^T[b, j] = <r_s_b, z_j>
            dps = mmp.tile([BL, B], F32, tag="mm")
            for c in range(NC_E):
                nc.tensor.matmul(
                    out=dps[:],
                    lhsT=rst[:, c * BL : (c + 1) * BL],
                    rhs=ztf[:, c * B : (c + 1) * B],
                    start=(c == 0), stop=(c == NC_E - 1),
                )
            # relu(1 - pos + D), mask-weighted sum
            zero_t = prs.tile([BL, B], F32, tag="zero_t")
            nc.vector.memset(zero_t[:], 0.0)
            relu_t = prs.tile([BL, B], F32, tag="relu_t")
            nc.vector.scalar_tensor_tensor(
                out=relu_t[:], in0=dps[:], scalar=om_pos[:], in1=zero_t[:],
                op0=AluOp.add, op1=AluOp.max,
            )
            mrg = prs.tile([BL, 1], F32, tag="mrg")
            nc.vector.scalar_tensor_tensor(
                out=scr[0:BL, 0:B], in0=relu_t[:], scalar=1.0, in1=mt[:],
                op0=AluOp.mult, op1=AluOp.mult, accum_out=mrg[:],
            )
            mps = mmp.tile([1, 1], F32, tag="mm")
            nc.tensor.matmul(out=mps[:], lhsT=onesf[0:BL, 0:1], rhs=mrg[:],
                             start=True, stop=True)
            mrg_sb = prs.tile([1, 1], F32, tag="mrg_sb")
            nc.scalar.copy(mrg_sb[:], mps[:])
            nc.sync.dma_start(out=mg_out, in_=mrg_sb[:])

    nc.compile()
    return nc


def _prep_inputs(x, W_att, b_att, W_red, b_red, aspect_W, neg_idx):
    x = np.ascontiguousarray(x, dtype=np.float32)
    M = np.zeros((B, B), dtype=np.float32)
    np.add.at(M, (neg_idx.reshape(-1),
                  np.repeat(np.arange(B), neg_idx.shape[1])), 1.0)
    ident = np.eye(128, dtype=np.float32)
    common = {
        "w_att16": np.ascontiguousarray(W_att, dtype=np.float16),
        "b_att16": np.asarray(b_att, dtype=np.float16).reshape(1, E),
        "w_red": np.ascontiguousarray(W_red, dtype=np.float32),
        "b_red_row": np.asarray(b_red, dtype=np.float32).reshape(1, A),
        "aspect_w": np.ascontiguousarray(aspect_W, dtype=np.float32),
        "ones_q32": np.full((128, 32), 1.0 / S, dtype=np.float32),
        "ones_f32": np.ones((128, 128), dtype=np.float32),
        "ones_16": np.ones((128, 128), dtype=np.float16),
        "ident_f32": ident,
        "ident_16": ident.astype(np.float16),
    }
    in_maps = []
    for c in range(N_CORES):
        m = dict(common)
        m["x_loc"] = np.ascontiguousarray(x[c * BL : (c + 1) * BL])
        m["m_t"] = np.ascontiguousarray(M[:, c * BL : (c + 1) * BL].T)
        in_maps.append(m)
    return in_maps


def kernel(x, W_att, b_att, W_red, b_red, aspect_W, neg_idx, _trace=False):
    if "nc" not in _CACHED:
        _CACHED["nc"] = build_program()
    nc = _CACHED["nc"]
    in_maps = _prep_inputs(x, W_att, b_att, W_red, b_red, aspect_W, neg_idx)
    res = bass_utils.run_bass_kernel_spmd(
        nc, in_maps, core_ids=list(range(N_CORES)), trace=_trace
    )
    _CACHED["last_result"] = res
    z = np.concatenate([res.results[c]["z_loc"] for c in range(N_CORES)], axis=0)
    recon = np.concatenate(
        [res.results[c]["recon_loc"] for c in range(N_CORES)], axis=0
    )
    margin = sum(float(res.results[c]["margin_loc"][0, 0]) for c in range(N_CORES))
    reg = float(res.results[0]["reg_out"][0, 0])
    loss = np.float32(reg + margin / (B * 10))
    return (z, recon, loss)


# revision 7
# speedup vs baseline: 1.0066x; 1.0066x over previous
"""Trainium2 Bass kernel for nn_AutoEncoder_1700807049822.

Data-parallel over batch: 8 cores x 32 batches. Per core:
  X [s=128, (b=32, e=768)] f32 in SBUF (12.6 MB), loaded in 8 groups of 4 batches.
  Q mean      : col-tiled f32 matmuls (ones lhsT, M=32-replicated rows)
  q transpose : PE transposes (fp16)
  QW = qW     : fp16 matmul per supergroup of 8 batches (W_att streamed)
  replicate   : K=1 row-tiled fp16 matmuls -> qW broadcast in PSUM
  scores      : DVE scalar_tensor_tensor fused mult+reduce (f32)
  softmax     : exp on ACT (no max-subtract; scores are O(10)), colsum via
                ones-matmul, normalization folded into the z PSUM evacuation
  z           : col-tiled f32 matmuls (exp-weights, M=32-replicated)
  comp/recon  : small f32 matmuls; loss tail with a zT AllGather and a
                host-prepared negative-count mask (handles duplicate indices)
Outputs per core: z rows, recon rows, margin partial sum, reg. Host glue sums
margin partials and adds reg.
"""

import numpy as np

import concourse.bass as bass
import concourse.tile as tile
from concourse import bacc, mybir
from concourse import bass_utils

F32 = mybir.dt.float32
F16 = mybir.dt.float16

N_CORES = 8
B, S, E, A = 256, 128, 768, 30
BL = B // N_CORES          # 32 local batches
NG = 8                     # batch groups per core
GB = BL // NG              # 4 batches per group
NC_E = E // 128            # 6 e-chunks
AluOp = mybir.AluOpType
Act = mybir.ActivationFunctionType

_CACHED = {}


def build_program(no_cc=False):
    nc = bacc.Bacc("TRN2", target_bir_lowering=False, debug=False,
                   num_devices=N_CORES)

    # ---- DRAM I/O ----
    x_d = nc.dram_tensor("x_loc", [BL, S, E], F32, kind="ExternalInput").ap()
    w16_d = nc.dram_tensor("w_att16", [E, E], F16, kind="ExternalInput").ap()
    batt16_d = nc.dram_tensor("b_att16", [1, E], F16, kind="ExternalInput").ap()
    wred_d = nc.dram_tensor("w_red", [E, A], F32, kind="ExternalInput").ap()
    bred_d = nc.dram_tensor("b_red_row", [1, A], F32, kind="ExternalInput").ap()
    asp_d = nc.dram_tensor("aspect_w", [A, E], F32, kind="ExternalInput").ap()
    mt_d = nc.dram_tensor("m_t", [BL, B], F32, kind="ExternalInput").ap()
    onesq_d = nc.dram_tensor("ones_q32", [128, 32], F16, kind="ExternalInput").ap()
    onesf_d = nc.dram_tensor("ones_f32", [128, 128], F32, kind="ExternalInput").ap()
    ones16_d = nc.dram_tensor("ones_16", [128, 128], F16, kind="ExternalInput").ap()
    i32_d = nc.dram_tensor("ident_f32", [128, 128], F32, kind="ExternalInput").ap()
    i16_d = nc.dram_tensor("ident_16", [128, 128], F16, kind="ExternalInput").ap()

    z_out = nc.dram_tensor("z_loc", [BL, E], F32, kind="ExternalOutput").ap()
    rc_out = nc.dram_tensor("recon_loc", [BL, E], F32, kind="ExternalOutput").ap()
    mg_out = nc.dram_tensor("margin_loc", [1, 1], F32, kind="ExternalOutput").ap()
    rg_out = nc.dram_tensor("reg_out", [1, 1], F32, kind="ExternalOutput").ap()

    with tile.TileContext(nc) as tc:
        with (
            tc.tile_pool(name="xp", bufs=NG) as xp,
            tc.tile_pool(name="cst", bufs=1) as cst,
            tc.tile_pool(name="wk", bufs=2) as wk,
            tc.tile_pool(name="prs", bufs=1) as prs,
            tc.tile_pool(name="mm", bufs=2, space="PSUM") as mmp,
            tc.tile_pool(name="qb", bufs=2, space="PSUM") as qbp,
            tc.tile_pool(name="dram", bufs=1, space="DRAM") as dram,
        ):
            # ---- constants / weights ----
            w16 = cst.tile([128, NC_E * E], F16, tag="w16")
            nc.sync.dma_start(
                out=w16[:],
                in_=w16_d.rearrange("(k p) e -> p k e", p=128),
            )
            batt16 = cst.tile([1, E], F16, tag="batt16")
            nc.sync.dma_start(out=batt16[:], in_=batt16_d)
            wred = cst.tile([128, NC_E * A], F32, tag="wred")
            nc.sync.dma_start(
                out=wred[:], in_=wred_d.rearrange("(k p) a -> p k a", p=128)
            )
            bred = cst.tile([1, A], F32, tag="bred")
            nc.sync.dma_start(out=bred[:], in_=bred_d)
            asp = cst.tile([A, E], F32, tag="asp")
            nc.sync.dma_start(out=asp[:], in_=asp_d)
            mt = cst.tile([BL, B], F32, tag="mt")
            nc.sync.dma_start(out=mt[:], in_=mt_d)
            onesq = cst.tile([128, 32], F16, tag="onesq")
            nc.sync.dma_start(out=onesq[:], in_=onesq_d)
            onesf = cst.tile([128, 128], F32, tag="onesf")
            nc.sync.dma_start(out=onesf[:], in_=onesf_d)
            ones16 = cst.tile([128, 128], F16, tag="ones16")
            nc.sync.dma_start(out=ones16[:], in_=ones16_d)
            ident = cst.tile([128, 128], F32, tag="ident")
            nc.sync.dma_start(out=ident[:], in_=i32_d)
            ident16 = cst.tile([128, 128], F16, tag="ident16")
            nc.sync.dma_start(out=ident16[:], in_=i16_d)

            # ---- persistent working tiles ----
            negc = prs.tile([128, 1], F32, tag="negc")
            nc.vector.memset(negc[:], -12.0)
            scoresT = prs.tile([128, BL], F32, tag="scoresT")
            exp_sT = prs.tile([128, BL], F32, tag="exp_sT")
            recip_row = prs.tile([1, BL], F32, tag="recip_row")
            zT = prs.tile([128, NC_E * BL], F32, tag="zT")
            z_bp = prs.tile([BL, E], F32, tag="z_bp")
            scr = prs.tile([128, E], F32, tag="scr")        # STT dump
            scr2 = prs.tile([128, E], F32, tag="scr2")      # misc dump
            tnt = prs.tile([128, NC_E * A], F32, tag="tnt")  # T_n^T

            # ---- X loads (one tile per group for precise deps) ----
            xg = []
            for g in range(NG):
                xt = xp.tile([128, GB * E], F16, tag="x")
                nc.gpsimd.dma_start(
                    out=xt[:],
                    in_=x_d[g * GB : (g + 1) * GB].rearrange("b s e -> s b e"),
                )
                xg.append(xt)

            # =========== regularizer (independent; overlaps DMA) ===========
            ss_a = prs.tile([A, 1], F32, tag="ss_a")
            nc.scalar.activation(out=scr2[0:A, :], in_=asp[:], func=Act.Square,
                                 accum_out=ss_a[:])
            inv_a = prs.tile([A, 1], F32, tag="inv_a")
            nc.vector.reciprocal(inv_a[:], ss_a[:])
            nc.scalar.activation(out=inv_a[:], in_=inv_a[:], func=Act.Sqrt)
            tn = prs.tile([A, E], F32, tag="tn")
            nc.vector.tensor_scalar_mul(tn[:], asp[:], inv_a[:])
            for c in range(NC_E):
                tp = mmp.tile([128, A], F32, tag="mm")
                nc.tensor.transpose(tp[:], tn[:, c * 128 : (c + 1) * 128],
                                    ident[0:A, 0:A])
                nc.scalar.copy(tnt[:, c * A : (c + 1) * A], tp[:])
            gps = mmp.tile([A, A], F32, tag="mm")
            for c in range(NC_E):
                nc.tensor.matmul(
                    out=gps[:],
                    lhsT=tnt[:, c * A : (c + 1) * A],
                    rhs=tnt[:, c * A : (c + 1) * A],
                    start=(c == 0),
                    stop=(c == NC_E - 1),
                )
            gm = prs.tile([A, A], F32, tag="gm")
            nc.vector.tensor_sub(gm[:], gps[:], ident[0:A, 0:A])
            grow = prs.tile([A, 1], F32, tag="grow")
            nc.vector.scalar_tensor_tensor(
                out=scr2[0:A, 0:A], in0=gm[:], scalar=1.0, in1=gm[:],
                op0=AluOp.mult, op1=AluOp.mult, accum_out=grow[:],
            )
            rps = mmp.tile([1, 1], F32, tag="mm")
            nc.tensor.matmul(out=rps[:], lhsT=onesf[0:A, 0:1], rhs=grow[:],
                             start=True, stop=True)
            reg_sb = prs.tile([1, 1], F32, tag="reg_sb")
            nc.scalar.activation(out=reg_sb[:], in_=rps[:], func=Act.Sqrt)
            nc.sync.dma_start(out=rg_out, in_=reg_sb[:])

            # =========== main pipeline ===========
            NSG = NG // 2  # supergroups of 8 batches
            for sg in range(NSG):
                qt16 = wk.tile([128, NC_E * 2 * GB], F16, tag="qt16")
                for t in range(2):
                    g = 2 * sg + t
                    # --- Q mean: col-tiled, M=32 replicated rows ---
                    qps = mmp.tile([128, E], F32, tag="mm")
                    for j in range(GB):
                        for (o, n) in ((0, 512), (512, 256)):
                            nc.tensor.matmul(
                                out=qps[32 * j : 32 * (j + 1), o : o + n],
                                lhsT=onesq[:],
                                rhs=xg[g][:, j * E + o : j * E + o + n],
                                start=True, stop=True,
                                tile_position=(0, 32 * j),
                            )
                    q16 = wk.tile([128, E], F16, tag="q16")
                    nc.scalar.copy(q16[:], qps[:])
                    # --- q transposes (fp16) ---
                    qtp = mmp.tile([128, E], F16, tag="mm")
                    for c in range(NC_E):
                        nc.tensor.transpose(
                            qtp[:, c * 128 : (c + 1) * 128],
                            q16[:, c * 128 : (c + 1) * 128],
                            ident16[:],
                        )
                    # evac cols {0,32,64,96} of each chunk
                    nc.scalar.copy(
                        qt16[:].rearrange("p (c b) -> p c b", b=2 * GB)[:, :, t * GB:(t + 1) * GB],
                        qtp[:].rearrange("p (c x) -> p c x", x=128)[:, :, ::32],
                    )
                # --- QW for the supergroup (fp16) ---
                qwps = mmp.tile([8, E], F32, tag="mm")
                for c in range(NC_E):
                    for (o, n) in ((0, 512), (512, 256)):
                        nc.tensor.matmul(
                            out=qwps[:, o : o + n],
                            lhsT=qt16[:].rearrange("p (c b) -> p c b", b=2 * GB)[:, c, :],
                            rhs=w16[:, c * E + o : c * E + o + n],
                            start=(c == 0), stop=False,
                        )
                for (o, n) in ((0, 512), (512, 256)):
                    nc.tensor.matmul(out=qwps[:, o : o + n],
                                     lhsT=ones16[0:1, 0:8],
                                     rhs=batt16[:, o : o + n],
                                     start=False, stop=True)
                qw8 = wk.tile([8, E], F16, tag="qw8")
                nc.scalar.copy(qw8[:], qwps[:])
                qw_sp = wk.tile([128, 2 * E], F16, tag="qw_sp")
                for t in range(2):
                    nc.sync.dma_start(
                        out=qw_sp[::32, t * E : (t + 1) * E],
                        in_=qw8[GB * t : GB * (t + 1), :],
                    )
                # --- replicate + scores per batch ---
                for t in range(2):
                    g = 2 * sg + t
                    for j in range(GB):
                        b = g * GB + j
                        qbps = qbp.tile([128, E], F32, tag="qb")
                        for (o, n) in ((0, 512), (512, 256)):
                            nc.tensor.matmul(
                                out=qbps[:, o : o + n],
                                lhsT=ones16[32 * j : 32 * j + 1, :],
                                rhs=qw_sp[32 * j : 32 * j + 1,
                                          t * E + o : t * E + o + n],
                                start=True, stop=True,
                                tile_position=(32 * j, 0),
                            )
                        nc.vector.scalar_tensor_tensor(
                            out=scr[:],
                            in0=xg[g][:, j * E : (j + 1) * E],
                            scalar=1.0,
                            in1=qbps[:],
                            op0=AluOp.mult, op1=AluOp.mult,
                            accum_out=scoresT[:, b : b + 1],
                        )
                # --- softmax + z per group ---
                for t in range(2):
                    g = 2 * sg + t
                    gc = slice(g * GB, (g + 1) * GB)
                    # shift by -12 so exp fits fp16 (cancels in softmax)
                    nc.scalar.activation(out=exp_sT[:, gc], in_=scoresT[:, gc],
                                         func=Act.Exp, bias=negc[:])
                    csps = mmp.tile([1, GB], F32, tag="mm")
                    nc.tensor.matmul(out=csps[:], lhsT=onesf[:, 0:1],
                                     rhs=exp_sT[:, gc], start=True, stop=True)
                    nc.vector.reciprocal(recip_row[:, gc], csps[:])
                    sc_col = wk.tile([128, 1], F32, tag="sc_col")
                    nc.vector.memset(sc_col[:], 1.0)
                    nc.sync.dma_start(out=sc_col[::32, :],
                                      in_=recip_row[0:1, gc])
                    exp_rep = wk.tile([128, GB, 32], F16, tag="exp_rep")
                    nc.vector.tensor_copy(
                        exp_rep[:],
                        exp_sT[:, gc].unsqueeze(2).broadcast_to([128, GB, 32]),
                    )
                    zps = mmp.tile([128, E], F32, tag="mm")
                    for j in range(GB):
                        for (o, n) in ((0, 512), (512, 256)):
                            nc.tensor.matmul(
                                out=zps[32 * j : 32 * (j + 1), o : o + n],
                                lhsT=exp_rep[:, j, :],
                                rhs=xg[g][:, j * E + o : j * E + o + n],
                                start=True, stop=True,
                                tile_position=(0, 32 * j),
                            )
                    z_sb = wk.tile([128, E], F32, tag="z_sb")
                    nc.scalar.activation(out=z_sb[:], in_=zps[:], func=Act.Copy,
                                         scale=sc_col[:])
                    nc.sync.dma_start(out=z_out[gc, :], in_=z_sb[::32, :])
                    nc.sync.dma_start(out=z_bp[gc, :], in_=z_sb[::32, :])
                    ztp = mmp.tile([128, E], F32, tag="mm")
                    for c in range(NC_E):
                        nc.tensor.transpose(
                            ztp[:, c * 128 : (c + 1) * 128],
                            z_sb[:, c * 128 : (c + 1) * 128],
                            ident[:],
                        )
                    nc.scalar.copy(
                        zT[:].rearrange("p (c b) -> p c b", b=BL)[:, :, gc],
                        ztp[:].rearrange("p (c x) -> p c x", x=128)[:, :, ::32],
                    )

            # =========== loss tail ===========
            # comp logits = z @ W_red + b_red   -> [BL, A] (b on partitions)
            cpps = mmp.tile([BL, A], F32, tag="mm")
            for c in range(NC_E):
                nc.tensor.matmul(
                    out=cpps[:],
                    lhsT=zT[:, c * BL : (c + 1) * BL],
                    rhs=wred[:, c * A : (c + 1) * A],
                    start=(c == 0), stop=False,
                )
            nc.tensor.matmul(out=cpps[:], lhsT=onesf[0:1, 0:BL], rhs=bred[:],
                             start=False, stop=True)
            exp_cp = prs.tile([BL, A], F32, tag="exp_cp")
            den = prs.tile([BL, 1], F32, tag="den")
            nc.scalar.activation(out=exp_cp[:], in_=cpps[:], func=Act.Exp,
                                 accum_out=den[:])
            rden = prs.tile([BL, 1], F32, tag="rden")
            nc.vector.reciprocal(rden[:], den[:])
            # compT via PE transpose
            ctps = mmp.tile([A, BL], F32, tag="mm")
            nc.tensor.transpose(ctps[:], exp_cp[:], ident[0:BL, 0:BL])
            compT = prs.tile([A, BL], F32, tag="compT")
            nc.scalar.copy(compT[:], ctps[:])
            # recon rows (scale by 1/den at evac)
            rcps = mmp.tile([BL, E], F32, tag="mm")
            for (o, n) in ((0, 512), (512, 256)):
                nc.tensor.matmul(
                    out=rcps[:, o : o + n],
                    lhsT=compT[:],
                    rhs=asp[:, o : o + n],
                    start=True, stop=True,
                )
            recon_sb = prs.tile([BL, E], F32, tag="recon_sb")
            nc.scalar.activation(out=recon_sb[:], in_=rcps[:], func=Act.Copy,
                                 scale=rden[:])
            nc.sync.dma_start(out=rc_out, in_=recon_sb[:])
            # r_s = recon / ||recon||
            ssr = prs.tile([BL, 1], F32, tag="ssr")
            nc.scalar.activation(out=scr2[0:BL, :], in_=recon_sb[:],
                                 func=Act.Square, accum_out=ssr[:])
            isq = prs.tile([BL, 1], F32, tag="isq")
            nc.vector.reciprocal(isq[:], ssr[:])
            nc.scalar.activation(out=isq[:], in_=isq[:], func=Act.Sqrt)
            rsc = prs.tile([BL, E], F32, tag="rsc")
            nc.vector.tensor_scalar_mul(rsc[:], recon_sb[:], isq[:])
            # pos = <z, r_s> ; om_pos = 1 - pos
            pos = prs.tile([BL, 1], F32, tag="pos")
            nc.vector.scalar_tensor_tensor(
                out=scr[0:BL, :], in0=z_bp[:], scalar=1.0, in1=rsc[:],
                op0=AluOp.mult, op1=AluOp.mult, accum_out=pos[:],
            )
            om_pos = prs.tile([BL, 1], F32, tag="om_pos")
            nc.vector.tensor_scalar(
                out=om_pos[:], in0=pos[:], scalar1=-1.0, scalar2=1.0,
                op0=AluOp.mult, op1=AluOp.add,
            )
            # r_s^T via PE transposes
            rst = prs.tile([128, NC_E * BL], F32, tag="rst")
            rtps = mmp.tile([128, BL], F32, tag="mm")
            for c in range(NC_E):
                nc.tensor.transpose(rtps[:], rsc[:, c * 128 : (c + 1) * 128],
                                    ident[0:BL, 0:BL])
                nc.scalar.copy(rst[:, c * BL : (c + 1) * BL], rtps[:])
            # AllGather zT
            zt_in = dram.tile([128, NC_E * BL], F32)
            zt_all = dram.tile([N_CORES, 128, NC_E * BL], F32)
            nc.sync.dma_start(out=zt_in[:], in_=zT[:])
            if no_cc:
                for cc in range(N_CORES):
                    nc.sync.dma_start(out=zt_all[cc], in_=zt_in[:])
            else:
                nc.gpsimd.collective_compute(
                    "AllGather", AluOp.bypass,
                    replica_groups=[list(range(N_CORES))],
                    ins=[zt_in[:].opt()], outs=[zt_all[:].opt()],
                )
            ztf = prs.tile([128, NC_E * B], F32, tag="ztf")
            for cc in range(N_CORES):
                nc.sync.dma_start(
                    out=ztf[:].rearrange("p (c j) -> p c j", j=B)[:, :, cc * BL:(cc + 1) * BL],
                    in_=zt_all[cc],
                )
            # D^T[b, j] = <r_s_b, z_j>
            dps = mmp.tile([BL, B], F32, tag="mm")
            for c in range(NC_E):
                nc.tensor.matmul(
                    out=dps[:],
                    lhsT=rst[:, c * BL : (c + 1) * BL],
                    rhs=ztf[:, c * B : (c + 1) * B],
                    start=(c == 0), stop=(c == NC_E - 1),
                )
            # relu(1 - pos + D), mask-weighted sum
            zero_t = prs.tile([BL, B], F32, tag="zero_t")
            nc.vector.memset(zero_t[:], 0.0)
            relu_t = prs.tile([BL, B], F32, tag="relu_t")
            nc.vector.scalar_tensor_tensor(
                out=relu_t[:], in0=dps[:], scalar=om_pos[:], in1=zero_t[:],
                op0=AluOp.add, op1=AluOp.max,
            )
            mrg = prs.tile([BL, 1], F32, tag="mrg")
            nc.vector.scalar_tensor_tensor(
                out=scr[0:BL, 0:B], in0=relu_t[:], scalar=1.0, in1=mt[:],
                op0=AluOp.mult, op1=AluOp.mult, accum_out=mrg[:],
            )
            mps = mmp.tile([1, 1], F32, tag="mm")
            nc.tensor.matmul(out=mps[:], lhsT=onesf[0:BL, 0:1], rhs=mrg[:],
                             start=True, stop=True)
            mrg_sb = prs.tile([1, 1], F32, tag="mrg_sb")
            nc.scalar.copy(mrg_sb[:], mps[:])
            nc.sync.dma_start(out=mg_out, in_=mrg_sb[:])

    nc.compile()
    return nc


def _prep_inputs(x, W_att, b_att, W_red, b_red, aspect_W, neg_idx):
    x = np.ascontiguousarray(x, dtype=np.float32)
    M = np.zeros((B, B), dtype=np.float32)
    np.add.at(M, (neg_idx.reshape(-1),
                  np.repeat(np.arange(B), neg_idx.shape[1])), 1.0)
    ident = np.eye(128, dtype=np.float32)
    common = {
        "w_att16": np.ascontiguousarray(W_att, dtype=np.float16),
        "b_att16": np.asarray(b_att, dtype=np.float16).reshape(1, E),
        "w_red": np.ascontiguousarray(W_red, dtype=np.float32),
        "b_red_row": np.asarray(b_red, dtype=np.float32).reshape(1, A),
        "aspect_w": np.ascontiguousarray(aspect_W, dtype=np.float32),
        "ones_q32": np.full((128, 32), 1.0 / S, dtype=np.float16),
        "ones_f32": np.ones((128, 128), dtype=np.float32),
        "ones_16": np.ones((128, 128), dtype=np.float16),
        "ident_f32": ident,
        "ident_16": ident.astype(np.float16),
    }
    in_maps = []
    for c in range(N_CORES):
        m = dict(common)
        m["x_loc"] = np.ascontiguousarray(x[c * BL : (c + 1) * BL])
        m["m_t"] = np.ascontiguousarray(M[:, c * BL : (c + 1) * BL].T)
        in_maps.append(m)
    return in_maps


def kernel(x, W_att, b_att, W_red, b_red, aspect_W, neg_idx, _trace=False):
    if "nc" not in _CACHED:
        _CACHED["nc"] = build_program()
    nc = _CACHED["nc"]
    in_maps = _prep_inputs(x, W_att, b_att, W_red, b_red, aspect_W, neg_idx)
    res = bass_utils.run_bass_kernel_spmd(
        nc, in_maps, core_ids=list(range(N_CORES)), trace=_trace
    )
    _CACHED["last_result"] = res
    z = np.concatenate([res.results[c]["z_loc"] for c in range(N_CORES)], axis=0)
    recon = np.concatenate(
        [res.results[c]["recon_loc"] for c in range(N_CORES)], axis=0
    )
    margin = sum(float(res.results[c]["margin_loc"][0, 0]) for c in range(N_CORES))
    reg = float(res.results[0]["reg_out"][0, 0])
    loss = np.float32(reg + margin / (B * 10))
    return (z, recon, loss)
